# revision 11
# baseline (speedup 1.0000x reference)
"""Fused BiMambaBlock device kernel (one SPMD launch on 8 cores).

Core c handles (dir, batch, head-group): dir=c>>2, batch=(c>>1)&1, hg=c&1.
Per core: in-proj GEMMs (causal conv folded in as 4 token-shifted weight
copies), chunked SSD scan (Q=128, segsum via K=1 PE matmuls + clamped exp),
gating, folded out-proj partial. Host combines partials (the RMSNorm scale
commutes through the out matmul).
"""
import sys
for p in ("/opt/pypackages", "/opt/trn_rl_repo"):
    if p not in sys.path:
        sys.path.insert(0, p)

from contextlib import ExitStack

import numpy as np
import ml_dtypes

import concourse.bass as bass
import concourse.bacc as bacc
import concourse.mybir as mybir
import concourse.tile as tile
from concourse.bass import AP

BF16 = mybir.dt.bfloat16
F32 = mybir.dt.float32
NPBF16 = ml_dtypes.bfloat16
AF = mybir.ActivationFunctionType
ALU = mybir.AluOpType

L = 2048
Q = 128
NCH = 16
H = 8          # local heads
KD = 512       # d_model
NEG = -100.0   # mask offset for t < s


def fap(t, offset_cols, dims):
    """AP over 2D view of tile t: keeps partition dim, replaces free dims.
    dims: [[step, count], ...] in elements; offset_cols in elements."""
    full = t[:, :]
    return AP(full.tensor, full.offset + offset_cols, [full.ap[0]] + dims)


def rowstride(t):
    """Per-partition element stride of a 2D tile view."""
    return t[:, :].ap[0][0]


def build_core_program():
    nc = bacc.Bacc(None, target_bir_lowering=False, debug=False)
    xnT = nc.dram_tensor("xnT", [KD, 3 + L], BF16, kind="ExternalInput")
    wx = nc.dram_tensor("wx", [4, KD, 512], BF16, kind="ExternalInput")
    wz = nc.dram_tensor("wz", [KD, 512], BF16, kind="ExternalInput")
    wbc = nc.dram_tensor("wbc", [4, KD, 128], BF16, kind="ExternalInput")
    wdt = nc.dram_tensor("wdt", [KD, 8], BF16, kind="ExternalInput")
    w2 = nc.dram_tensor("w2", [KD, 512], BF16, kind="ExternalInput")
    cbx = nc.dram_tensor("cbx", [1, 512], BF16, kind="ExternalInput")
    cbbc = nc.dram_tensor("cbbc", [128, 1], F32, kind="ExternalInput")
    dtb = nc.dram_tensor("dtb", [8, 1], F32, kind="ExternalInput")
    negA = nc.dram_tensor("negA", [128, 1], F32, kind="ExternalInput")
    dfull = nc.dram_tensor("dfull", [128, 512], BF16, kind="ExternalInput")
    maskoff = nc.dram_tensor("maskoff", [128, 512], F32, kind="ExternalInput")
    ones1f = nc.dram_tensor("ones1f", [1, 128], F32, kind="ExternalInput")
    ones1b = nc.dram_tensor("ones1b", [1, 128], BF16, kind="ExternalInput")
    ones128b = nc.dram_tensor("ones128b", [128, 1], BF16, kind="ExternalInput")
    identf = nc.dram_tensor("identf", [128, 128], F32, kind="ExternalInput")
    identb = nc.dram_tensor("identb", [128, 128], BF16, kind="ExternalInput")

    acdram = nc.dram_tensor("acdram", [NCH, 1024], F32, kind="Internal")
    decdram = nc.dram_tensor("decdram", [NCH, 512], BF16, kind="Internal")
    pout = nc.dram_tensor("pout", [512, L], F32, kind="ExternalOutput")
    ssq = nc.dram_tensor("ssq", [1, L], F32, kind="ExternalOutput")

    with tile.TileContext(nc) as tc, ExitStack() as ctx:
        pers = ctx.enter_context(tc.tile_pool(name="pers", bufs=1))
        work = ctx.enter_context(tc.tile_pool(name="work", bufs=2))
        pre = ctx.enter_context(tc.tile_pool(name="pre", bufs=4))
        pgemm = ctx.enter_context(tc.tile_pool(name="pgemm", bufs=3, space="PSUM"))
        pT = ctx.enter_context(tc.tile_pool(name="pT", bufs=2, space="PSUM"))
        pYst = ctx.enter_context(tc.tile_pool(name="pYst", bufs=1, space="PSUM"))
        pHD = ctx.enter_context(tc.tile_pool(name="pHD", bufs=2, space="PSUM"))

        # ---- load inputs ----
        xnT_sb = pers.tile([128, 4, 3 + L], BF16, tag="xnT")
        nc.sync.dma_start(xnT_sb[:, :, :], xnT.rearrange("(k p) t -> p k t", p=128))
        wx_sb = pers.tile([128, 4, 4, 512], BF16, tag="wx")
        nc.sync.dma_start(wx_sb[:, :, :, :], wx.rearrange("j (k p) f -> p j k f", p=128))
        wz_sb = pers.tile([128, 4, 512], BF16, tag="wz")
        nc.sync.dma_start(wz_sb[:, :, :], wz.rearrange("(k p) f -> p k f", p=128))
        wbc_sb = pers.tile([128, 4, 4, 128], BF16, tag="wbc")
        nc.sync.dma_start(wbc_sb[:, :, :, :], wbc.rearrange("j (k p) f -> p j k f", p=128))
        wdt_sb = pers.tile([128, 4, 8], BF16, tag="wdt")
        nc.sync.dma_start(wdt_sb[:, :, :], wdt.rearrange("(k p) f -> p k f", p=128))
        w2_sb = pers.tile([128, 4, 512], BF16, tag="w2")
        nc.sync.dma_start(w2_sb[:, :, :], w2.rearrange("(k p) f -> p k f", p=128))
        cbx_sb = pers.tile([1, 512], BF16, tag="cbx")
        nc.sync.dma_start(cbx_sb[:, :], cbx[:, :])
        cbbc_sb = pers.tile([128, 1], F32, tag="cbbc")
        nc.sync.dma_start(cbbc_sb[:, :], cbbc[:, :])
        dtb_sb = pers.tile([8, 1], F32, tag="dtb")
        nc.sync.dma_start(dtb_sb[:, :], dtb[:, :])
        negA_sb = pers.tile([128, 1], F32, tag="negA")
        nc.sync.dma_start(negA_sb[:, :], negA[:, :])
        dfull_sb = pers.tile([128, 512], BF16, tag="dfull")
        nc.sync.dma_start(dfull_sb[:, :], dfull[:, :])
        mask_sb = pers.tile([128, 512], F32, tag="maskoff")
        nc.sync.dma_start(mask_sb[:, :], maskoff[:, :])
        ones1f_sb = pers.tile([1, 128], F32, tag="ones1f")
        nc.sync.dma_start(ones1f_sb[:, :], ones1f[:, :])
        ones1b_sb = pers.tile([1, 128], BF16, tag="ones1b")
        nc.sync.dma_start(ones1b_sb[:, :], ones1b[:, :])
        ones128b_sb = pers.tile([128, 1], BF16, tag="ones128b")
        nc.sync.dma_start(ones128b_sb[:, :], ones128b[:, :])
        identf_sb = pers.tile([128, 128], F32, tag="identf")
        nc.sync.dma_start(identf_sb[:, :], identf[:, :])
        identb_sb = pers.tile([128, 128], BF16, tag="identb")
        nc.sync.dma_start(identb_sb[:, :], identb[:, :])

        # ---- persistent intermediates ----
        BCc_sb = pers.tile([128, L], BF16, tag="BCc")   # B rows 0:64, C raw 64:128
        C_sb = pers.tile([64, L], BF16, tag="C")        # C at base partition 0
        B_tm = pers.tile([128, NCH, 64], BF16, tag="Btm")
        dtfm_sb = pers.tile([8, L], F32, tag="dtfm")
        X_sb = pers.tile([128, NCH, 512], BF16, tag="X")
        siluz_sb = pers.tile([128, NCH, 512], BF16, tag="siluz")
        v_fm = pers.tile([128, 4, L], BF16, tag="vfm")
        H_bf = pers.tile([64, 512], BF16, tag="Hbf")

        dtscan = pers.tile([128, 128], F32, tag="dtscan")
        ascan = pers.tile([128, 128], F32, tag="ascan")
        zeros128 = pers.tile([128, 128], F32, tag="zeros128")
        Acum = pers.tile([128, 128], F32, tag="Acum")
        negAcumT = pers.tile([128, 128], F32, tag="negAcumT")
        eAT = pers.tile([128, 128], F32, tag="eAT")
        dtT_sb = pers.tile([128, 128], BF16, tag="dtT")
        wgtp = pers.tile([128, 128], F32, tag="wgtp")
        wgtT_sb = pers.tile([128, 128], BF16, tag="wgtT")
        decrow = pers.tile([1, 128], F32, tag="decrow")

        nc.vector.memset(zeros128, 0.0)
        nc.vector.memset(H_bf, 0.0)
        # ACT warm-up reads of DMA-loaded bias tiles: later activations then
        # only carry single-engine waits (ACT ISA allows one sync wait).
        scr1 = pers.tile([8, 1], F32, tag="scr1")
        nc.scalar.copy(scr1, dtb_sb[:, 0:1])
        scr2 = pers.tile([128, 1], F32, tag="scr2")
        nc.scalar.copy(scr2, cbbc_sb[:, 0:1])

        # ============ phase B: dt + BC feature-major GEMMs ============
        for c4 in range(4):
            psDT = pgemm.tile([8, 512], F32, tag="psg")
            for k in range(4):
                nc.tensor.matmul(psDT, wdt_sb[:, k, :],
                                 xnT_sb[:, k, 3 + c4 * 512: 3 + (c4 + 1) * 512],
                                 start=(k == 0), stop=(k == 3))
            dte = work.tile([8, 512], F32, tag="dte")
            nc.scalar.activation(dte, psDT, AF.Exp, bias=dtb_sb[:, 0:1])
            nc.vector.tensor_scalar_add(dte, dte, 1.0)
            nc.scalar.activation(dtfm_sb[:, c4 * 512:(c4 + 1) * 512], dte, AF.Ln)
        for c4 in range(4):
            psBC = pgemm.tile([128, 512], F32, tag="psg")
            first = True
            for j in range(4):
                for k in range(4):
                    nc.tensor.matmul(psBC, wbc_sb[:, j, k, :],
                                     xnT_sb[:, k, c4 * 512 + j: c4 * 512 + j + 512],
                                     start=first, stop=(j == 3 and k == 3))
                    first = False
            nc.scalar.activation(BCc_sb[:, c4 * 512:(c4 + 1) * 512], psBC,
                                 AF.Silu, bias=cbbc_sb[:, 0:1])
        nc.sync.dma_start(C_sb[:, :], BCc_sb[64:128, :])

        # ============ phase C: X / z token-major GEMMs ============
        for c in range(NCH):
            t0 = c * Q
            psX = pgemm.tile([128, 512], F32, tag="psg")
            first = True
            for j in range(4):
                for k in range(4):
                    nc.tensor.matmul(psX, xnT_sb[:, k, t0 + j: t0 + j + 128],
                                     wx_sb[:, j, k, :], start=first, stop=False)
                    first = False
            nc.tensor.matmul(psX, ones1b_sb[0:1, :], cbx_sb[0:1, :],
                             start=False, stop=True)
            nc.scalar.activation(X_sb[:, c, :], psX, AF.Silu)
            psZ = pgemm.tile([128, 512], F32, tag="psg")
            for k in range(4):
                nc.tensor.matmul(psZ, xnT_sb[:, k, 3 + t0: 3 + t0 + 128],
                                 wz_sb[:, k, :], start=(k == 0), stop=(k == 3))
            nc.scalar.activation(siluz_sb[:, c, :], psZ, AF.Silu)

        # ============ phase D: dt pipeline ============
        # dt [8 part, (c,t)] -> [(h,c) part, t]  (flat element orders match)
        dts_full = dtscan[:, :]
        dtf_full = dtfm_sb[:, :]
        nc.sync.dma_start(
            AP(dts_full.tensor, dts_full.offset,
               [[rowstride(dtscan), 128], [1, 128]]),
            AP(dtf_full.tensor, dtf_full.offset,
               [[rowstride(dtfm_sb), 8], [1, 2048]]))
        nc.vector.tensor_scalar_mul(ascan, dtscan, negA_sb[:, 0:1])
        nc.vector.tensor_tensor_scan(Acum, ascan, zeros128, 0.0,
                                     op0=ALU.add, op1=ALU.add)
        psTa = pT.tile([128, 128], F32, tag="psT")
        nc.tensor.transpose(psTa, Acum[:, :], identf_sb[:, :])
        nc.vector.tensor_scalar_mul(negAcumT, psTa, -1.0)
        nc.scalar.activation(eAT, psTa, AF.Exp)
        psTd = pT.tile([128, 128], F32, tag="psT")
        nc.tensor.transpose(psTd, dtscan[:, :], identf_sb[:, :])
        nc.vector.tensor_copy(dtT_sb, psTd)
        nc.vector.tensor_scalar(wgtp, Acum, Acum[:, 127:128], None,
                                op0=ALU.subtract)
        nc.scalar.activation(wgtp, wgtp, AF.Exp, scale=-1.0)
        psTw = pT.tile([128, 128], F32, tag="psT")
        nc.tensor.transpose(psTw, wgtp[:, :], identf_sb[:, :])
        nc.vector.tensor_copy(wgtT_sb, psTw)
        nc.sync.dma_start(decrow[:, :], eAT[127:128, :])
        # whole-Acum bounce to DRAM in (c, h, t) order for broadcast reads
        acd_full = acdram[:, :]
        nc.sync.dma_start(
            AP(acd_full.tensor, acd_full.offset,
               [[128, 8], [1024, 16], [1, 128]]),
            Acum[:, :])
        # dec expanded (h -> 64-wide) per chunk, bf16, one bounce
        decall = pers.tile([1, 8192], BF16, tag="decall")
        for c in range(NCH):
            nc.vector.tensor_copy(decall[0:1, c * 512:(c + 1) * 512],
                                  fap(decrow, c, [[16, 8], [0, 64]]))
        nc.sync.dma_start(decdram[:, :], decall[:, :].rearrange("p (c f) -> p c f", c=16))
        for c in range(NCH):
            psTb = pT.tile([128, 128], BF16, tag="psT")
            nc.tensor.transpose(psTb[:, 0:64], BCc_sb[0:64, c * Q:(c + 1) * Q],
                                identb_sb[0:64, 0:64])
            nc.vector.tensor_copy(B_tm[:, c, :], psTb[:, 0:64])

        # ============ phase E: per-chunk scan ============
        for c in range(NCH):
            t0 = c * Q
            psG = pT.tile([128, 128], F32, tag="psT")
            nc.tensor.matmul(psG, BCc_sb[0:64, t0:t0 + Q], C_sb[:, t0:t0 + Q],
                             start=True, stop=True)
            Gt_sb = work.tile([128, 128], BF16, tag="Gt")
            nc.vector.tensor_copy(Gt_sb, psG)
            # Xdt / Xw
            Xdt = work.tile([128, 512], BF16, tag="Xdt")
            nc.vector.tensor_mul(Xdt, X_sb[:, c, :],
                                 fap(dtT_sb, c, [[16, 8], [0, 64]]))
            Xw = work.tile([128, 512], BF16, tag="Xw")
            nc.vector.tensor_mul(Xw, Xdt, fap(wgtT_sb, c, [[16, 8], [0, 64]]))
            # segment matrices: 2 groups x 4 heads
            Mt = work.tile([128, 1024], BF16, tag="Mt")
            for g in range(2):
                dbase = pre.tile([128, 512], F32, tag="dbase")
                adf = acdram[c:c + 1, g * 512:(g + 1) * 512]
                nc.sync.dma_start(
                    dbase[:, :],
                    AP(adf.tensor, adf.offset, [[0, 128]] + adf.ap[1:]))
                t1 = work.tile([128, 512], F32, tag="t1")
                nc.vector.scalar_tensor_tensor(
                    t1, dbase, 0.0,
                    fap(negAcumT, g * 64 + c, [[16, 4], [0, 128]]),
                    op0=ALU.add, op1=ALU.add)
                t2 = work.tile([128, 512], F32, tag="t2")
                nc.vector.scalar_tensor_tensor(
                    t2, t1, 0.0, mask_sb[:, :], op0=ALU.min, op1=ALU.add)
                Eg = work.tile([128, 512], BF16, tag="Eg")
                nc.scalar.activation(Eg, t2, AF.Exp)
                nc.vector.tensor_mul(Mt[:, g * 512:(g + 1) * 512], Eg,
                                     fap(Gt_sb, 0, [[0, 4], [1, 128]]))
            # Ystate + Yintra
            psYst = pYst.tile([128, 512], F32, tag="psYst")
            nc.tensor.matmul(psYst, C_sb[:, t0:t0 + Q], H_bf[:, :],
                             start=True, stop=True)
            psYin = pgemm.tile([128, 512], F32, tag="psg")
            for h in range(H):
                nc.tensor.matmul(psYin[:, h * 64:(h + 1) * 64],
                                 Mt[:, h * 128:(h + 1) * 128],
                                 Xdt[:, h * 64:(h + 1) * 64],
                                 start=True, stop=True)
            yst = work.tile([128, 512], BF16, tag="yst")
            nc.vector.tensor_mul(yst, psYst, fap(eAT, c, [[16, 8], [0, 64]]))
            y1 = work.tile([128, 512], BF16, tag="y1")
            nc.vector.tensor_add(y1, psYin, yst)
            xd = work.tile([128, 512], BF16, tag="xd")
            nc.vector.tensor_mul(xd, X_sb[:, c, :], dfull_sb[:, :])
            y2 = work.tile([128, 512], BF16, tag="y2")
            nc.vector.tensor_add(y2, y1, xd)
            v_tm = work.tile([128, 512], BF16, tag="vtm")
            nc.vector.tensor_mul(v_tm, y2, siluz_sb[:, c, :])
            v2scr = work.tile([128, 512], BF16, tag="v2scr")
            ssq_col = work.tile([128, 1], F32, tag="ssqcol")
            nc.scalar.activation(v2scr, v_tm, AF.Square, accum_out=ssq_col)
            nc.sync.dma_start(ssq[0:1, t0:t0 + Q], ssq_col[:, 0:1])
            # state update
            decrep = pre.tile([64, 512], BF16, tag="decrep")
            ddf = decdram[c:c + 1, :]
            nc.sync.dma_start(
                decrep[:, :],
                AP(ddf.tensor, ddf.offset, [[0, 64]] + ddf.ap[1:]))
            psH = pHD.tile([64, 512], F32, tag="psHD")
            nc.tensor.matmul(psH, B_tm[:, c, :], Xw, start=True, stop=True)
            Htmp = work.tile([64, 512], BF16, tag="Htmp")
            nc.vector.tensor_mul(Htmp, H_bf, decrep)
            nc.vector.tensor_add(H_bf, Htmp, psH)
            # v to feature-major
            for f in range(4):
                psTv = pT.tile([128, 128], BF16, tag="psT")
                nc.tensor.transpose(psTv, v_tm[:, f * 128:(f + 1) * 128],
                                    identb_sb[:, :])
                nc.any.tensor_copy(v_fm[:, f, t0:t0 + Q], psTv)

        # ============ phase F: out-proj partial + ssq ============
        for mt in range(4):
            for c4 in range(4):
                psO = pgemm.tile([128, 512], F32, tag="psg")
                for k in range(4):
                    nc.tensor.matmul(psO, w2_sb[:, k, mt * 128:(mt + 1) * 128],
                                     v_fm[:, k, c4 * 512:(c4 + 1) * 512],
                                     start=(k == 0), stop=(k == 3))
                ostage = work.tile([128, 512], F32, tag="ostage")
                nc.scalar.copy(ostage, psO)
                nc.sync.dma_start(pout[mt * 128:(mt + 1) * 128,
                                       c4 * 512:(c4 + 1) * 512], ostage)

    nc.compile()
    return nc


# ---------------- host-side input prep ----------------

def prep_core_inputs(inputs, d, b, hg, xn_all):
    """Build the DRAM input dict for one core. xn_all: (B, L, 512) f32."""
    pref = 'fwd' if d == 0 else 'bwd'
    in_w = np.asarray(inputs[f'{pref}_in_w'], np.float32)
    conv_w = np.asarray(inputs[f'{pref}_conv_w'], np.float32)
    conv_b = np.asarray(inputs[f'{pref}_conv_b'], np.float32)
    dt_bias = np.asarray(inputs[f'{pref}_dt_bias'], np.float32)
    A_log = np.asarray(inputs[f'{pref}_A_log'], np.float32)
    Dp = np.asarray(inputs[f'{pref}_D'], np.float32)
    norm_w = np.asarray(inputs[f'{pref}_norm_w'], np.float32)
    out_w = np.asarray(inputs[f'{pref}_out_w'], np.float32)
    proj_w = np.asarray(inputs['proj_w'], np.float32)

    xn = xn_all[b] if d == 0 else xn_all[b, ::-1]

    Wz = in_w[hg * 512: hg * 512 + 512]
    Wx = in_w[1024 + hg * 512: 1024 + hg * 512 + 512]
    Wbc = in_w[2048:2176]
    Wdt = in_w[2176 + hg * 8: 2176 + hg * 8 + 8]
    cw_x = conv_w[hg * 512: hg * 512 + 512]
    cw_bc = conv_w[1024:1152]
    cb_x = conv_b[hg * 512: hg * 512 + 512]
    cb_bc = conv_b[1024:1152]

    xnT = np.zeros((KD, 3 + L), NPBF16)
    xnT[:, 3:] = np.ascontiguousarray(xn.T).astype(NPBF16)

    wx_a = np.empty((4, KD, 512), NPBF16)
    wbc_a = np.empty((4, KD, 128), NPBF16)
    for j in range(4):
        wx_a[j] = (Wx * cw_x[:, j:j + 1]).T.astype(NPBF16)
        wbc_a[j] = (Wbc * cw_bc[:, j:j + 1]).T.astype(NPBF16)

    W2 = proj_w[:, d * 512:(d + 1) * 512] @ out_w          # (512, 1024)
    w2_a = (W2[:, hg * 512: hg * 512 + 512] *
            norm_w[None, hg * 512: hg * 512 + 512]).T.astype(NPBF16)

    negA_a = np.empty((128, 1), np.float32)
    for h in range(H):
        negA_a[h * 16:(h + 1) * 16, 0] = -np.exp(A_log[hg * 8 + h])
    dfull_a = np.repeat(Dp[hg * 8: hg * 8 + 8], 64)[None, :].repeat(128, 0)

    sidx = np.arange(128)[:, None]
    tidx = np.arange(128)[None, :]
    mask1 = np.where(tidx >= sidx, 0.0, NEG).astype(np.float32)
    maskoff_a = np.tile(mask1, (1, 4))

    return {
        "xnT": xnT,
        "wx": wx_a,
        "wz": Wz.T.astype(NPBF16).copy(),
        "wbc": wbc_a,
        "wdt": Wdt.T.astype(NPBF16).copy(),
        "w2": w2_a.copy(),
        "cbx": cb_x[None, :].astype(NPBF16).copy(),
        "cbbc": np.concatenate([cb_bc, np.zeros(0, np.float32)])[:, None].copy(),
        "dtb": dt_bias[hg * 8: hg * 8 + 8][:, None].astype(np.float32).copy(),
        "negA": negA_a,
        "dfull": dfull_a.astype(NPBF16).copy(),
        "maskoff": maskoff_a,
        "ones1f": np.ones((1, 128), np.float32),
        "ones1b": np.ones((1, 128), NPBF16),
        "ones128b": np.ones((128, 1), NPBF16),
        "identf": np.eye(128, dtype=np.float32),
        "identb": np.eye(128).astype(NPBF16),
    }


# ---------------- SPMD launch + host combine ----------------

from concourse.bass_utils import run_bass_kernel_spmd

NCORES = 8
EPS = 1e-5
_prog_cache = {}
TRACE = False
EXEC_NS = []


def _get_prog():
    if "prog" not in _prog_cache:
        _prog_cache["prog"] = build_core_program()
    return _prog_cache["prog"]


def kernel(**inputs):
    x = np.asarray(inputs["x"], np.float32)            # (2, 2048, 512)
    B, L_, D = x.shape
    xn_all = x * (1.0 / np.sqrt(np.mean(x * x, -1, keepdims=True) + EPS)) \
        * np.asarray(inputs["norm_w"], np.float32)

    nc = _get_prog()
    in_maps = []
    for core in range(NCORES):
        d, b, hg = core >> 2, (core >> 1) & 1, core & 1
        in_maps.append(prep_core_inputs(inputs, d, b, hg, xn_all))

    try:
        res = run_bass_kernel_spmd(nc, in_maps, core_ids=list(range(NCORES)),
                                   trace=TRACE)
    except ModuleNotFoundError:
        res = run_bass_kernel_spmd(nc, in_maps, core_ids=list(range(NCORES)),
                                   trace=False)
    EXEC_NS.append(res.exec_time_ns)

    proj_b = np.asarray(inputs["proj_b"], np.float32)
    out = x + proj_b[None, None, :]
    for d in range(2):
        for b in range(B):
            c0 = d * 4 + b * 2
            p = res.results[c0]["pout"].astype(np.float32).T + \
                res.results[c0 + 1]["pout"].astype(np.float32).T
            sq = res.results[c0]["ssq"].astype(np.float32)[0] + \
                res.results[c0 + 1]["ssq"].astype(np.float32)[0]
            s = 1.0 / np.sqrt(sq / 1024.0 + EPS)
            yd = s[:, None] * p
            if d == 1:
                yd = yd[::-1]
            out[b] += yd
    return out.astype(np.float32)


# revision 13
# speedup vs baseline: 1.1355x; 1.1355x over previous
"""Fused BiMambaBlock device kernel (one SPMD launch on 8 cores).

Core c handles (dir, batch, head-group): dir=c>>2, batch=(c>>1)&1, hg=c&1.
Per core: in-proj GEMMs (causal conv folded in as 4 token-shifted weight
copies), chunked SSD scan (Q=128, segsum via K=1 PE matmuls + clamped exp),
gating, folded out-proj partial. Host combines partials (the RMSNorm scale
commutes through the out matmul).
"""
import sys
for p in ("/opt/pypackages", "/opt/trn_rl_repo"):
    if p not in sys.path:
        sys.path.insert(0, p)

from contextlib import ExitStack

import numpy as np
import ml_dtypes

import concourse.bass as bass
import concourse.bacc as bacc
import concourse.mybir as mybir
import concourse.tile as tile
from concourse.bass import AP

BF16 = mybir.dt.bfloat16
F32 = mybir.dt.float32
NPBF16 = ml_dtypes.bfloat16
AF = mybir.ActivationFunctionType
ALU = mybir.AluOpType

L = 2048
Q = 128
NCH = 16
H = 8          # local heads
KD = 512       # d_model
NEG = -100.0   # mask offset for t < s


def fap(t, offset_cols, dims):
    """AP over 2D view of tile t: keeps partition dim, replaces free dims.
    dims: [[step, count], ...] in elements; offset_cols in elements."""
    full = t[:, :]
    return AP(full.tensor, full.offset + offset_cols, [full.ap[0]] + dims)


def rowstride(t):
    """Per-partition element stride of a 2D tile view."""
    return t[:, :].ap[0][0]


def build_core_program():
    nc = bacc.Bacc(None, target_bir_lowering=False, debug=False)
    xnT = nc.dram_tensor("xnT", [KD, 3 + L], BF16, kind="ExternalInput")
    wx = nc.dram_tensor("wx", [4, KD, 512], BF16, kind="ExternalInput")
    wz = nc.dram_tensor("wz", [KD, 512], BF16, kind="ExternalInput")
    wbc = nc.dram_tensor("wbc", [4, KD, 128], BF16, kind="ExternalInput")
    wdt = nc.dram_tensor("wdt", [KD, 8], BF16, kind="ExternalInput")
    w2 = nc.dram_tensor("w2", [KD, 512], BF16, kind="ExternalInput")
    cbx = nc.dram_tensor("cbx", [1, 512], BF16, kind="ExternalInput")
    cbbc = nc.dram_tensor("cbbc", [128, 1], F32, kind="ExternalInput")
    dtb = nc.dram_tensor("dtb", [8, 1], F32, kind="ExternalInput")
    negA = nc.dram_tensor("negA", [128, 1], F32, kind="ExternalInput")
    dfull = nc.dram_tensor("dfull", [128, 512], BF16, kind="ExternalInput")
    maskoff = nc.dram_tensor("maskoff", [128, 512], F32, kind="ExternalInput")
    ones1f = nc.dram_tensor("ones1f", [1, 128], F32, kind="ExternalInput")
    ones1b = nc.dram_tensor("ones1b", [1, 128], BF16, kind="ExternalInput")
    ones128b = nc.dram_tensor("ones128b", [128, 1], BF16, kind="ExternalInput")
    identf = nc.dram_tensor("identf", [128, 128], F32, kind="ExternalInput")
    identb = nc.dram_tensor("identb", [128, 128], BF16, kind="ExternalInput")

    acdram = nc.dram_tensor("acdram", [NCH, 1024], F32, kind="Internal")
    decdram = nc.dram_tensor("decdram", [NCH, 512], BF16, kind="Internal")
    pout = nc.dram_tensor("pout", [512, L], F32, kind="ExternalOutput")
    ssq = nc.dram_tensor("ssq", [1, L], F32, kind="ExternalOutput")

    with tile.TileContext(nc) as tc, ExitStack() as ctx:
        pers = ctx.enter_context(tc.tile_pool(name="pers", bufs=1))
        work = ctx.enter_context(tc.tile_pool(name="work", bufs=2))
        pre = ctx.enter_context(tc.tile_pool(name="pre", bufs=4))
        pgemm = ctx.enter_context(tc.tile_pool(name="pgemm", bufs=3, space="PSUM"))
        pT = ctx.enter_context(tc.tile_pool(name="pT", bufs=2, space="PSUM"))
        pYst = ctx.enter_context(tc.tile_pool(name="pYst", bufs=1, space="PSUM"))
        pHD = ctx.enter_context(tc.tile_pool(name="pHD", bufs=2, space="PSUM"))

        # ---- load inputs ----
        xnT_sb = pers.tile([128, 4, 3 + L], BF16, tag="xnT")
        nc.sync.dma_start(xnT_sb[:, :, :], xnT.rearrange("(k p) t -> p k t", p=128))
        wx_sb = pers.tile([128, 4, 4, 512], BF16, tag="wx")
        nc.sync.dma_start(wx_sb[:, :, :, :], wx.rearrange("j (k p) f -> p j k f", p=128))
        wz_sb = pers.tile([128, 4, 512], BF16, tag="wz")
        nc.sync.dma_start(wz_sb[:, :, :], wz.rearrange("(k p) f -> p k f", p=128))
        wbc_sb = pers.tile([128, 4, 4, 128], BF16, tag="wbc")
        nc.sync.dma_start(wbc_sb[:, :, :, :], wbc.rearrange("j (k p) f -> p j k f", p=128))
        wdt_sb = pers.tile([128, 4, 8], BF16, tag="wdt")
        nc.sync.dma_start(wdt_sb[:, :, :], wdt.rearrange("(k p) f -> p k f", p=128))
        w2_sb = pers.tile([128, 4, 512], BF16, tag="w2")
        nc.sync.dma_start(w2_sb[:, :, :], w2.rearrange("(k p) f -> p k f", p=128))
        cbx_sb = pers.tile([1, 512], BF16, tag="cbx")
        nc.sync.dma_start(cbx_sb[:, :], cbx[:, :])
        cbbc_sb = pers.tile([128, 1], F32, tag="cbbc")
        nc.sync.dma_start(cbbc_sb[:, :], cbbc[:, :])
        dtb_sb = pers.tile([8, 1], F32, tag="dtb")
        nc.sync.dma_start(dtb_sb[:, :], dtb[:, :])
        negA_sb = pers.tile([128, 1], F32, tag="negA")
        nc.sync.dma_start(negA_sb[:, :], negA[:, :])
        dfull_sb = pers.tile([128, 512], BF16, tag="dfull")
        nc.sync.dma_start(dfull_sb[:, :], dfull[:, :])
        mask_sb = pers.tile([128, 512], F32, tag="maskoff")
        nc.sync.dma_start(mask_sb[:, :], maskoff[:, :])
        ones1f_sb = pers.tile([1, 128], F32, tag="ones1f")
        nc.sync.dma_start(ones1f_sb[:, :], ones1f[:, :])
        ones1b_sb = pers.tile([1, 128], BF16, tag="ones1b")
        nc.sync.dma_start(ones1b_sb[:, :], ones1b[:, :])
        ones128b_sb = pers.tile([128, 1], BF16, tag="ones128b")
        nc.sync.dma_start(ones128b_sb[:, :], ones128b[:, :])
        identf_sb = pers.tile([128, 128], F32, tag="identf")
        nc.sync.dma_start(identf_sb[:, :], identf[:, :])
        identb_sb = pers.tile([128, 128], BF16, tag="identb")
        nc.sync.dma_start(identb_sb[:, :], identb[:, :])

        # ---- persistent intermediates ----
        BCc_sb = pers.tile([128, L], BF16, tag="BCc")   # B rows 0:64, C raw 64:128
        C_sb = pers.tile([64, L], BF16, tag="C")        # C at base partition 0
        B_tm = pers.tile([128, NCH, 64], BF16, tag="Btm")
        dtfm_sb = pers.tile([8, L], F32, tag="dtfm")
        X_sb = pers.tile([128, NCH, 512], BF16, tag="X")
        siluz_sb = pers.tile([128, NCH, 512], BF16, tag="siluz")
        v_fm = pers.tile([128, 4, L], BF16, tag="vfm")
        H_bf = pers.tile([64, 512], BF16, tag="Hbf")

        dtscan = pers.tile([128, 128], F32, tag="dtscan")
        ascan = pers.tile([128, 128], F32, tag="ascan")
        zeros128 = pers.tile([128, 128], F32, tag="zeros128")
        Acum = pers.tile([128, 128], F32, tag="Acum")
        negAcumT = pers.tile([128, 128], F32, tag="negAcumT")
        eAT = pers.tile([128, 128], F32, tag="eAT")
        dtT_sb = pers.tile([128, 128], BF16, tag="dtT")
        wgtp = pers.tile([128, 128], F32, tag="wgtp")
        wgtT_sb = pers.tile([128, 128], BF16, tag="wgtT")
        decrow = pers.tile([1, 128], F32, tag="decrow")

        nc.vector.memset(zeros128, 0.0)
        nc.vector.memset(H_bf, 0.0)
        # ACT warm-up reads of DMA-loaded bias tiles: later activations then
        # only carry single-engine waits (ACT ISA allows one sync wait).
        scr1 = pers.tile([8, 1], F32, tag="scr1")
        nc.scalar.copy(scr1, dtb_sb[:, 0:1])
        scr2 = pers.tile([128, 1], F32, tag="scr2")
        nc.scalar.copy(scr2, cbbc_sb[:, 0:1])

        # ============ phase B: dt + BC feature-major GEMMs ============
        for c4 in range(4):
            psDT = pgemm.tile([8, 512], F32, tag="psg")
            for k in range(4):
                nc.tensor.matmul(psDT, wdt_sb[:, k, :],
                                 xnT_sb[:, k, 3 + c4 * 512: 3 + (c4 + 1) * 512],
                                 start=(k == 0), stop=(k == 3))
            dte = work.tile([8, 512], F32, tag="dte")
            nc.scalar.activation(dte, psDT, AF.Exp, bias=dtb_sb[:, 0:1])
            nc.vector.tensor_scalar_add(dte, dte, 1.0)
            nc.scalar.activation(dtfm_sb[:, c4 * 512:(c4 + 1) * 512], dte, AF.Ln)
        for c4 in range(4):
            psBC = pgemm.tile([128, 512], F32, tag="psg")
            first = True
            for j in range(4):
                for k in range(4):
                    nc.tensor.matmul(psBC, wbc_sb[:, j, k, :],
                                     xnT_sb[:, k, c4 * 512 + j: c4 * 512 + j + 512],
                                     start=first, stop=(j == 3 and k == 3))
                    first = False
            nc.scalar.activation(BCc_sb[:, c4 * 512:(c4 + 1) * 512], psBC,
                                 AF.Silu, bias=cbbc_sb[:, 0:1])
        nc.sync.dma_start(C_sb[:, :], BCc_sb[64:128, :])

        # ============ phase C: X / z token-major GEMMs ============
        for c in range(NCH):
            t0 = c * Q
            psX = pgemm.tile([128, 512], F32, tag="psg")
            first = True
            for j in range(4):
                for k in range(4):
                    nc.tensor.matmul(psX, xnT_sb[:, k, t0 + j: t0 + j + 128],
                                     wx_sb[:, j, k, :], start=first, stop=False)
                    first = False
            nc.tensor.matmul(psX, ones1b_sb[0:1, :], cbx_sb[0:1, :],
                             start=False, stop=True)
            nc.scalar.activation(X_sb[:, c, :], psX, AF.Silu)
            psZ = pgemm.tile([128, 512], F32, tag="psg")
            for k in range(4):
                nc.tensor.matmul(psZ, xnT_sb[:, k, 3 + t0: 3 + t0 + 128],
                                 wz_sb[:, k, :], start=(k == 0), stop=(k == 3))
            nc.scalar.activation(siluz_sb[:, c, :], psZ, AF.Silu)

        # ============ phase D: dt pipeline ============
        # dt [8 part, (c,t)] -> [(h,c) part, t]  (flat element orders match)
        dts_full = dtscan[:, :]
        dtf_full = dtfm_sb[:, :]
        nc.sync.dma_start(
            AP(dts_full.tensor, dts_full.offset,
               [[rowstride(dtscan), 128], [1, 128]]),
            AP(dtf_full.tensor, dtf_full.offset,
               [[rowstride(dtfm_sb), 8], [1, 2048]]))
        nc.vector.tensor_scalar_mul(ascan, dtscan, negA_sb[:, 0:1])
        nc.vector.tensor_tensor_scan(Acum, ascan, zeros128, 0.0,
                                     op0=ALU.add, op1=ALU.add)
        psTa = pT.tile([128, 128], F32, tag="psT")
        nc.tensor.transpose(psTa, Acum[:, :], identf_sb[:, :])
        nc.vector.tensor_scalar_mul(negAcumT, psTa, -1.0)
        nc.scalar.activation(eAT, psTa, AF.Exp)
        psTd = pT.tile([128, 128], F32, tag="psT")
        nc.tensor.transpose(psTd, dtscan[:, :], identf_sb[:, :])
        nc.vector.tensor_copy(dtT_sb, psTd)
        nc.vector.tensor_scalar(wgtp, Acum, Acum[:, 127:128], None,
                                op0=ALU.subtract)
        nc.scalar.activation(wgtp, wgtp, AF.Exp, scale=-1.0)
        psTw = pT.tile([128, 128], F32, tag="psT")
        nc.tensor.transpose(psTw, wgtp[:, :], identf_sb[:, :])
        nc.vector.tensor_copy(wgtT_sb, psTw)
        nc.sync.dma_start(decrow[:, :], eAT[127:128, :])
        # whole-Acum bounce to DRAM in (c, h, t) order for broadcast reads
        acd_full = acdram[:, :]
        nc.sync.dma_start(
            AP(acd_full.tensor, acd_full.offset,
               [[128, 8], [1024, 16], [1, 128]]),
            Acum[:, :])
        # dec expanded (h -> 64-wide) per chunk, bf16, one bounce
        decall = pers.tile([1, 8192], BF16, tag="decall")
        for c in range(NCH):
            nc.vector.tensor_copy(decall[0:1, c * 512:(c + 1) * 512],
                                  fap(decrow, c, [[16, 8], [0, 64]]))
        nc.sync.dma_start(decdram[:, :], decall[:, :].rearrange("p (c f) -> p c f", c=16))
        for c in range(NCH):
            psTb = pT.tile([128, 128], BF16, tag="psT")
            nc.tensor.transpose(psTb[:, 0:64], BCc_sb[0:64, c * Q:(c + 1) * Q],
                                identb_sb[0:64, 0:64])
            nc.vector.tensor_copy(B_tm[:, c, :], psTb[:, 0:64])

        # ============ phase E: per-chunk scan ============
        for c in range(NCH):
            t0 = c * Q
            psG = pT.tile([128, 128], F32, tag="psT")
            nc.tensor.matmul(psG, BCc_sb[0:64, t0:t0 + Q], C_sb[:, t0:t0 + Q],
                             start=True, stop=True)
            Gt_sb = work.tile([128, 128], BF16, tag="Gt")
            nc.vector.tensor_copy(Gt_sb, psG)
            # Xdt / Xw
            Xdt = work.tile([128, 512], BF16, tag="Xdt")
            nc.vector.tensor_mul(Xdt, X_sb[:, c, :],
                                 fap(dtT_sb, c, [[16, 8], [0, 64]]))
            Xw = work.tile([128, 512], BF16, tag="Xw")
            nc.vector.tensor_mul(Xw, Xdt, fap(wgtT_sb, c, [[16, 8], [0, 64]]))
            # segment matrices: 2 groups x 4 heads
            Mt = work.tile([128, 1024], BF16, tag="Mt")
            for g in range(2):
                dbase = pre.tile([128, 512], F32, tag="dbase")
                adf = acdram[c:c + 1, g * 512:(g + 1) * 512]
                nc.sync.dma_start(
                    dbase[:, :],
                    AP(adf.tensor, adf.offset, [[0, 128]] + adf.ap[1:]))
                t1 = work.tile([128, 512], F32, tag="t1")
                nc.vector.scalar_tensor_tensor(
                    t1, dbase, 0.0,
                    fap(negAcumT, g * 64 + c, [[16, 4], [0, 128]]),
                    op0=ALU.add, op1=ALU.add)
                t2 = work.tile([128, 512], F32, tag="t2")
                nc.vector.scalar_tensor_tensor(
                    t2, t1, 0.0, mask_sb[:, :], op0=ALU.min, op1=ALU.add)
                Eg = work.tile([128, 512], BF16, tag="Eg")
                nc.scalar.activation(Eg, t2, AF.Exp)
                nc.vector.tensor_mul(Mt[:, g * 512:(g + 1) * 512], Eg,
                                     fap(Gt_sb, 0, [[0, 4], [1, 128]]))
            # Ystate + Yintra
            psYst = pYst.tile([128, 512], F32, tag="psYst")
            nc.tensor.matmul(psYst, C_sb[:, t0:t0 + Q], H_bf[:, :],
                             start=True, stop=True)
            psYin = pgemm.tile([128, 512], F32, tag="psg")
            for h in range(H):
                nc.tensor.matmul(psYin[:, h * 64:(h + 1) * 64],
                                 Mt[:, h * 128:(h + 1) * 128],
                                 Xdt[:, h * 64:(h + 1) * 64],
                                 start=True, stop=True)
            yst = work.tile([128, 512], BF16, tag="yst")
            nc.vector.tensor_mul(yst, psYst, fap(eAT, c, [[16, 8], [0, 64]]))
            y1 = work.tile([128, 512], BF16, tag="y1")
            nc.vector.tensor_add(y1, psYin, yst)
            xd = work.tile([128, 512], BF16, tag="xd")
            nc.vector.tensor_mul(xd, X_sb[:, c, :], dfull_sb[:, :])
            y2 = work.tile([128, 512], BF16, tag="y2")
            nc.vector.tensor_add(y2, y1, xd)
            v_tm = work.tile([128, 512], BF16, tag="vtm")
            nc.vector.tensor_mul(v_tm, y2, siluz_sb[:, c, :])
            v2scr = work.tile([128, 512], BF16, tag="v2scr")
            ssq_col = work.tile([128, 1], F32, tag="ssqcol")
            nc.scalar.activation(v2scr, v_tm, AF.Square, accum_out=ssq_col)
            nc.sync.dma_start(ssq[0:1, t0:t0 + Q], ssq_col[:, 0:1])
            # state update
            decrep = pre.tile([64, 512], BF16, tag="decrep")
            ddf = decdram[c:c + 1, :]
            nc.sync.dma_start(
                decrep[:, :],
                AP(ddf.tensor, ddf.offset, [[0, 64]] + ddf.ap[1:]))
            psH = pHD.tile([64, 512], F32, tag="psHD")
            nc.tensor.matmul(psH, B_tm[:, c, :], Xw, start=True, stop=True)
            Htmp = work.tile([64, 512], BF16, tag="Htmp")
            nc.vector.tensor_mul(Htmp, H_bf, decrep)
            nc.vector.tensor_add(H_bf, Htmp, psH)
            # v to feature-major
            for f in range(4):
                psTv = pT.tile([128, 128], BF16, tag="psT")
                nc.tensor.transpose(psTv, v_tm[:, f * 128:(f + 1) * 128],
                                    identb_sb[:, :])
                nc.any.tensor_copy(v_fm[:, f, t0:t0 + Q], psTv)

        # ============ phase F: out-proj partial + ssq ============
        for mt in range(4):
            for c4 in range(4):
                psO = pgemm.tile([128, 512], F32, tag="psg")
                for k in range(4):
                    nc.tensor.matmul(psO, w2_sb[:, k, mt * 128:(mt + 1) * 128],
                                     v_fm[:, k, c4 * 512:(c4 + 1) * 512],
                                     start=(k == 0), stop=(k == 3))
                ostage = work.tile([128, 512], F32, tag="ostage")
                nc.scalar.copy(ostage, psO)
                nc.sync.dma_start(pout[mt * 128:(mt + 1) * 128,
                                       c4 * 512:(c4 + 1) * 512], ostage)

    nc.compile()
    return nc


# ---------------- host-side input prep ----------------

def prep_core_inputs(inputs, d, b, hg, xn_all):
    """Build the DRAM input dict for one core. xn_all: (B, L, 512) f32."""
    pref = 'fwd' if d == 0 else 'bwd'
    in_w = np.asarray(inputs[f'{pref}_in_w'], np.float32)
    conv_w = np.asarray(inputs[f'{pref}_conv_w'], np.float32)
    conv_b = np.asarray(inputs[f'{pref}_conv_b'], np.float32)
    dt_bias = np.asarray(inputs[f'{pref}_dt_bias'], np.float32)
    A_log = np.asarray(inputs[f'{pref}_A_log'], np.float32)
    Dp = np.asarray(inputs[f'{pref}_D'], np.float32)
    norm_w = np.asarray(inputs[f'{pref}_norm_w'], np.float32)
    out_w = np.asarray(inputs[f'{pref}_out_w'], np.float32)
    proj_w = np.asarray(inputs['proj_w'], np.float32)

    xn = xn_all[b] if d == 0 else xn_all[b, ::-1]

    Wz = in_w[hg * 512: hg * 512 + 512]
    Wx = in_w[1024 + hg * 512: 1024 + hg * 512 + 512]
    Wbc = in_w[2048:2176]
    Wdt = in_w[2176 + hg * 8: 2176 + hg * 8 + 8]
    cw_x = conv_w[hg * 512: hg * 512 + 512]
    cw_bc = conv_w[1024:1152]
    cb_x = conv_b[hg * 512: hg * 512 + 512]
    cb_bc = conv_b[1024:1152]

    xnT = np.zeros((KD, 3 + L), NPBF16)
    xnT[:, 3:] = np.ascontiguousarray(xn.T).astype(NPBF16)

    wx_a = np.empty((4, KD, 512), NPBF16)
    wbc_a = np.empty((4, KD, 128), NPBF16)
    for j in range(4):
        wx_a[j] = (Wx * cw_x[:, j:j + 1]).T.astype(NPBF16)
        wbc_a[j] = (Wbc * cw_bc[:, j:j + 1]).T.astype(NPBF16)

    W2 = proj_w[:, d * 512:(d + 1) * 512] @ out_w          # (512, 1024)
    w2_a = (W2[:, hg * 512: hg * 512 + 512] *
            norm_w[None, hg * 512: hg * 512 + 512]).T.astype(NPBF16)

    negA_a = np.empty((128, 1), np.float32)
    for h in range(H):
        negA_a[h * 16:(h + 1) * 16, 0] = -np.exp(A_log[hg * 8 + h])
    dfull_a = np.repeat(Dp[hg * 8: hg * 8 + 8], 64)[None, :].repeat(128, 0)

    sidx = np.arange(128)[:, None]
    tidx = np.arange(128)[None, :]
    mask1 = np.where(tidx >= sidx, 0.0, NEG).astype(np.float32)
    maskoff_a = np.tile(mask1, (1, 4))

    return {
        "xnT": xnT,
        "wx": wx_a,
        "wz": Wz.T.astype(NPBF16).copy(),
        "wbc": wbc_a,
        "wdt": Wdt.T.astype(NPBF16).copy(),
        "w2": w2_a.copy(),
        "cbx": cb_x[None, :].astype(NPBF16).copy(),
        "cbbc": np.concatenate([cb_bc, np.zeros(0, np.float32)])[:, None].copy(),
        "dtb": dt_bias[hg * 8: hg * 8 + 8][:, None].astype(np.float32).copy(),
        "negA": negA_a,
        "dfull": dfull_a.astype(NPBF16).copy(),
        "maskoff": maskoff_a,
        "ones1f": np.ones((1, 128), np.float32),
        "ones1b": np.ones((1, 128), NPBF16),
        "ones128b": np.ones((128, 1), NPBF16),
        "identf": np.eye(128, dtype=np.float32),
        "identb": np.eye(128).astype(NPBF16),
    }


# ---------------- SPMD launch + host combine ----------------

from concourse.bass_utils import run_bass_kernel_spmd

NCORES = 8
EPS = 1e-5
_prog_cache = {}
TRACE = False
EXEC_NS = []


def _get_prog():
    if "prog" not in _prog_cache:
        _prog_cache["prog"] = build_core_program()
    return _prog_cache["prog"]


def kernel(**inputs):
    x = np.asarray(inputs["x"], np.float32)            # (2, 2048, 512)
    B, L_, D = x.shape
    xn_all = x * (1.0 / np.sqrt(np.mean(x * x, -1, keepdims=True) + EPS)) \
        * np.asarray(inputs["norm_w"], np.float32)

    nc = _get_prog()
    in_maps = []
    for core in range(NCORES):
        d, b, hg = core >> 2, (core >> 1) & 1, core & 1
        in_maps.append(prep_core_inputs(inputs, d, b, hg, xn_all))

    try:
        res = run_bass_kernel_spmd(nc, in_maps, core_ids=list(range(NCORES)),
                                   trace=TRACE)
    except ModuleNotFoundError:
        res = run_bass_kernel_spmd(nc, in_maps, core_ids=list(range(NCORES)),
                                   trace=False)
    EXEC_NS.append(res.exec_time_ns)

    proj_b = np.asarray(inputs["proj_b"], np.float32)
    out = x + proj_b[None, None, :]
    for d in range(2):
        for b in range(B):
            c0 = d * 4 + b * 2
            p = res.results[c0]["pout"].astype(np.float32).T + \
                res.results[c0 + 1]["pout"].astype(np.float32).T
            sq = res.results[c0]["ssq"].astype(np.float32)[0] + \
                res.results[c0 + 1]["ssq"].astype(np.float32)[0]
            s = 1.0 / np.sqrt(sq / 1024.0 + EPS)
            yd = s[:, None] * p
            if d == 1:
                yd = yd[::-1]
            out[b] += yd
    return out.astype(np.float32)
